# revision 16
# baseline (speedup 1.0000x reference)
"""3-layer GAT (GATConv x3, PyG-style) on Trainium2 across 8 NeuronCores.

v2 design. Destination nodes are 1D-partitioned across 8 cores (6250/core);
edges (self-loops appended) are sorted by dst into per-core blocks of 125
dst nodes, and within a block split into two index banks (int16 gather
indices only reach 32767) and padded to 128-edge "slots".

Per-edge data paths:
  - Layer 1 needs no gather and no collective: the host pre-gathers x[src]
    into per-slot transposed tiles (xe); the device matmuls each slot
    against W1_aug on the PE to get per-edge features directly.
  - Layers 2/3 gather rows of the AllGathered node table (768B / 256B rows)
    with dma_gather; table rows carry per-head [h(64) | 1.0] groups plus
    a_src logits, so the edge-value multiply produces both the weighted
    features and the softmax-denominator columns in one op.
  - a_dst logits are expanded dst->edge with small PE matmuls against
    host-shipped TRANSPOSED one-hot tiles (saT), not a DMA gather.
  - One-hot selection tiles (sa) ship from the host in bf16; the segment
    softmax + scatter-add is PE one-hot matmuls accumulated in PSUM.
  - exp(leaky(s)) = max(exp(0.2 s), exp(s)) via two ACT exps + one DVE max.
  - Epilogue normalization runs on ACT with a per-partition reciprocal
    scale; bias/relu/residual on DVE with contiguous access patterns.

Everything is self-contained: shapes/sharding hardcoded for the nn_GAT
problem (N=50000, E=800000, 128->4x64->4x64->64).
"""

import math
import os
import sys

import numpy as np

sys.path.insert(0, "/opt/trn_rl_repo")

import ml_dtypes

BF16 = ml_dtypes.bfloat16

# ----------------------------------------------------------------- problem
N_NODES = 50000
N_EDGES = 800000
IN_DIM = 128
HID = 64
HEADS = 4
OUT_DIM = 64
NEG_SLOPE = 0.2

N_CORES = 8
BANK = 25000  # int16 gather index bank split (must be <= 32768)
AG_CHUNK = 125  # rows per AllGather chunk (mesh regime)
GATHER_CHUNK = 1024  # max indices per dma_gather call
N_SWDGE_Q = 1  # SWDGE queues used round-robin for gathers
DMA_SCRATCH = 16384  # descriptor-ring carveout bytes/partition

ROW = {1: 384, 2: 128}  # gathered table row sizes (bf16 elems)


def _cfg_full():
    return dict(
        n=N_NODES,
        ncores=N_CORES,
        vpc=N_NODES // N_CORES,
        blk=125,
        bank=BANK,
    )


# ------------------------------------------------------------- host prep
def _wrap_idx(idx):
    """[n] int array -> [128, n//16] int16 in the SWDGE wrapped layout
    (position i lives at partition i%16, column i//16; replicated x8)."""
    n = idx.shape[0]
    assert n % 16 == 0
    w = np.asarray(idx, np.int16).reshape(n // 16, 16).T  # [16, n//16]
    return np.tile(w, (8, 1))  # [128, n//16]


def _remap(g, vpc, ncores):
    """Node id -> row in the chunk-interleaved AllGather output table."""
    rank, within = g // vpc, g % vpc
    chunk, i = within // AG_CHUNK, within % AG_CHUNK
    return chunk * (ncores * AG_CHUNK) + rank * AG_CHUNK + i


def build_host_data(x, edge_index, Ws, cfg):
    n = cfg["n"]
    ncores = cfg["ncores"]
    vpc = cfg["vpc"]
    blk = cfg["blk"]
    bank = cfg["bank"]
    nblk = vpc // blk
    assert nblk * blk == vpc and vpc * ncores == n

    src = np.concatenate([np.asarray(edge_index[0], np.int64), np.arange(n)])
    dst = np.concatenate([np.asarray(edge_index[1], np.int64), np.arange(n)])
    order = np.argsort(dst, kind="stable")
    src, dst = src[order], dst[order]
    srcR = _remap(src, vpc, ncores)

    gblk = dst // blk
    nb_all = ncores * nblk
    bstart = np.searchsorted(gblk, np.arange(nb_all))
    bend = np.searchsorted(gblk, np.arange(nb_all), side="right")

    # per-block-position slot counts, maxed over cores (SPMD program)
    S0s, S1s = [], []
    for cb in range(nblk):
        m0 = m1 = 1
        for c in range(ncores):
            b = c * nblk + cb
            s = srcR[bstart[b] : bend[b]]
            n0 = int((s < bank).sum())
            n1 = int(len(s) - n0)
            m0 = max(m0, math.ceil(max(n0, 1) / 128))
            m1 = max(m1, math.ceil(max(n1, 1) / 128))
        S0s.append(m0)
        S1s.append(m1)
    Sb = [a + b for a, b in zip(S0s, S1s)]
    OFF = np.concatenate([[0], np.cumsum(Sb)]).astype(int)
    TOT = int(OFF[-1])
    SMAX = max(Sb)

    xT_full = np.ascontiguousarray(np.asarray(x, np.float32).T).astype(BF16)

    per_core = []
    for c in range(ncores):
        eidx = np.zeros((128, TOT * 8), np.int16)
        sa = np.zeros((128, TOT * 128), BF16)
        saT = np.zeros((128, TOT * 128), BF16)
        xe = np.zeros((128, TOT * 128), BF16)
        for cb in range(nblk):
            b = c * nblk + cb
            lo, hi = bstart[b], bend[b]
            sR, sO = srcR[lo:hi], src[lo:hi]
            d = (dst[lo:hi] - b * blk).astype(np.int64)
            in0 = sR < bank
            co = int(OFF[cb])
            S0 = S0s[cb]
            for half, (sRh, sOh, dh, soff, scnt) in enumerate(
                [
                    (sR[in0], sO[in0], d[in0], 0, S0),
                    (sR[~in0] - bank, sO[~in0], d[~in0], S0, S1s[cb]),
                ]
            ):
                k = np.arange(len(sRh))
                part = k % 128
                cols = (co + soff + k // 128) * 128
                sa[part, cols + dh] = 1.0
                saT[dh, cols + part] = 1.0
                xe[:, cols + part] = xT_full[:, sOh]
                idx = np.zeros(scnt * 128, np.int16)
                idx[: len(sRh)] = sRh
                eidx[:, (co + soff) * 8 : (co + soff + scnt) * 8] = _wrap_idx(idx)
        xT = np.ascontiguousarray(xT_full[:, c * vpc : (c + 1) * vpc])
        per_core.append(dict(xT=xT, eidx=eidx, sa=sa, saT=saT, xe=xe))

    # ---- shared constants
    def headfold(W, a):
        # [fin, H*C] x [H, C] -> [fin, H] per-head logit weights
        H, C = a.shape
        return np.stack(
            [W[:, h * C : (h + 1) * C] @ a[h] for h in range(H)], axis=1
        )

    W1 = np.asarray(Ws["W1"], np.float32)
    W2 = np.asarray(Ws["W2"], np.float32)
    W3 = np.asarray(Ws["W3"], np.float32)
    As1 = headfold(W1, np.asarray(Ws["as1"], np.float32))
    Ad1 = headfold(W1, np.asarray(Ws["ad1"], np.float32))
    As2 = headfold(W2, np.asarray(Ws["as2"], np.float32))
    Ad2 = headfold(W2, np.asarray(Ws["ad2"], np.float32))
    As3 = headfold(W3, np.asarray(Ws["as3"], np.float32))
    Ad3 = headfold(W3, np.asarray(Ws["ad3"], np.float32))

    W1aug = np.concatenate([W1, As1], axis=1)  # [128, 260]
    W2aug = np.zeros((256, 268), np.float32)
    for h in range(4):
        W2aug[:, h * 65 : h * 65 + 64] = W2[:, h * 64 : (h + 1) * 64]
    W2aug[:, 260:264] = As2
    W2aug[:, 264:268] = Ad2
    W3aug = np.zeros((256, 68), np.float32)
    W3aug[:, 0:64] = W3
    W3aug[:, 65:66] = As3
    W3aug[:, 66:67] = Ad3

    consts = dict(
        W1aug=W1aug.astype(BF16),
        W1Ad=Ad1.astype(BF16),
        W2aug=W2aug.astype(BF16),
        W3aug=W3aug.astype(BF16),
        b1=np.tile(np.asarray(Ws["b1"], np.float32)[None, :], (128, 1)),
        b2=np.tile(np.asarray(Ws["b2"], np.float32)[None, :], (128, 1)),
        b3=np.tile(np.asarray(Ws["b3"], np.float32)[None, :], (128, 1)),
        ident=np.eye(128, dtype=np.float32).astype(BF16),
    )
    meta = dict(S0s=S0s, S1s=S1s, OFF=OFF.tolist(), TOT=TOT, SMAX=SMAX,
                nblk=nblk, **cfg)
    return per_core, consts, meta


# ------------------------------------------------------------ device build
def build_program(meta, consts):
    import concourse.bass as bass
    import concourse.mybir as mybir
    import concourse.tile as tile
    from concourse import bacc

    f32 = mybir.dt.float32
    bf16 = mybir.dt.bfloat16
    i16 = mybir.dt.int16
    Alu = mybir.AluOpType
    Act = mybir.ActivationFunctionType

    n = meta["n"]
    ncores = meta["ncores"]
    vpc = meta["vpc"]
    blk = meta["blk"]
    nblk = meta["nblk"]
    bank = meta["bank"]
    S0s, S1s, OFF = meta["S0s"], meta["S1s"], meta["OFF"]
    TOT, SMAX = meta["TOT"], meta["SMAX"]
    NT = math.ceil(vpc / 128)

    # per-layer static dims
    H_ = {0: 4, 1: 4, 2: 1}
    FH_ = {0: 256, 1: 256, 2: 64}
    MCOL = {0: 256, 1: 260, 2: 65}  # scatter rhs width
    HG = {0: 64, 1: 65, 2: 65}  # per-head stride in ps_sc
    ALS = {0: 256, 1: 260, 2: 65}  # a_src column offset in edge rows
    ROWL = {0: 260, 1: ROW[1], 2: ROW[2]}  # edge-row stride

    nc = bacc.Bacc(trn_type="TRN2", num_devices=ncores,
                   num_swdge_queues=N_SWDGE_Q,
                   dynamic_dma_scratch_size=DMA_SCRATCH)
    rg = [list(range(ncores))]

    # ---------------- I/O ----------------
    xT_in = nc.dram_tensor("xT", [128, vpc], bf16, kind="ExternalInput")
    eidx_in = nc.dram_tensor("eidx", [128, TOT * 8], i16, kind="ExternalInput")
    sa_in = nc.dram_tensor("sa", [128, TOT * 128], bf16, kind="ExternalInput")
    saT_in = nc.dram_tensor("saT", [128, TOT * 128], bf16, kind="ExternalInput")
    xe_in = nc.dram_tensor("xe", [128, TOT * 128], bf16, kind="ExternalInput")
    out3 = nc.dram_tensor("out3", [vpc, OUT_DIM], f32, kind="ExternalOutput")

    W1aug_t = nc.inline_tensor(consts["W1aug"], "W1aug")
    W1Ad_t = nc.inline_tensor(consts["W1Ad"], "W1Ad")
    W2aug_t = nc.inline_tensor(consts["W2aug"], "W2aug")
    W3aug_t = nc.inline_tensor(consts["W3aug"], "W3aug")
    b1_t = nc.inline_tensor(consts["b1"], "b1c")
    b2_t = nc.inline_tensor(consts["b2"], "b2c")
    b3_t = nc.inline_tensor(consts["b3"], "b3c")
    ident_t = nc.inline_tensor(consts["ident"], "identc")

    # internal DRAM
    tabs_in = {li: nc.dram_tensor(f"tab{li}_in", [vpc, ROW[li]], bf16)
               for li in (1, 2)}
    tabs = {li: nc.dram_tensor(f"tab{li}", [n, ROW[li]], bf16,
                               addr_space="Shared") for li in (1, 2)}
    aldb = {li: nc.dram_tensor(f"aldb{li}", [vpc, 4], bf16) for li in (0, 1, 2)}
    x1f = nc.dram_tensor("x1f", [vpc, 256], f32)
    xT2 = nc.dram_tensor("xT2", [256, vpc], bf16)
    xT3 = nc.dram_tensor("xT3", [256, vpc], bf16)
    xT_next = {0: xT2, 1: xT3}
    lhsT_srcs = {1: xT2, 2: xT3}

    AP = bass.AP

    def rd(ap, offset_elems, dims):
        return AP(ap.tensor, ap.offset + offset_elems,
                  [list(ap.ap[0])] + [list(d) for d in dims])

    with tile.TileContext(nc) as tc:
        with (
            tc.tile_pool(name="const", bufs=1) as cpool,
            tc.tile_pool(name="p1", bufs=3) as p1,
            tc.tile_pool(name="g", bufs=3) as gp,
            tc.tile_pool(name="e", bufs=2) as ep,
            tc.tile_pool(name="small", bufs=4) as sp,
            tc.tile_pool(name="psA", bufs=2, space="PSUM") as ppA,
            tc.tile_pool(name="psB", bufs=2, space="PSUM") as ppB,
            tc.tile_pool(name="psC", bufs=2, space="PSUM") as ppC,
            tc.tile_pool(name="psumT", bufs=2, space="PSUM") as ppT,
        ):
            ident_sb = cpool.tile([128, 128], bf16, tag="ident")
            nc.sync.dma_start(ident_sb[:], ident_t[:])
            bias_sb = []
            for li, bt in enumerate([b1_t, b2_t, b3_t]):
                b_sb = cpool.tile([128, bt.shape[1]], f32, tag=f"bias{li}")
                nc.sync.dma_start(b_sb[:], bt[:])
                bias_sb.append(b_sb)
            W1aug_sb = cpool.tile([128, 260], bf16, tag="w1aug")
            nc.sync.dma_start(W1aug_sb[:], W1aug_t[:])
            W1Ad_sb = cpool.tile([128, 4], bf16, tag="w1ad")
            nc.sync.dma_start(W1Ad_sb[:], W1Ad_t[:])
            W2aug_sb = []
            for k in range(2):
                w_sb = cpool.tile([128, 268], bf16, tag=f"w2aug{k}")
                nc.sync.dma_start(w_sb[:], W2aug_t[k * 128 : (k + 1) * 128, :])
                W2aug_sb.append(w_sb)
            W3aug_sb = []
            for k in range(2):
                w_sb = cpool.tile([128, 68], bf16, tag=f"w3aug{k}")
                nc.sync.dma_start(w_sb[:], W3aug_t[k * 128 : (k + 1) * 128, :])
                W3aug_sb.append(w_sb)

            # ---------------- phase 0: aldb[0] = x @ (W1 A_d) -------------
            def p0_tile(t):
                nt = min(128, vpc - t * 128)
                lw = p1.tile([128, 128], bf16, tag="lw")
                nc.sync.dma_start(lw[:, 0:nt],
                                  xT_in[:, t * 128 : t * 128 + nt])
                ps0 = ppB.tile([128, 268], f32, tag="psA")
                nc.tensor.matmul(ps0[0:nt, 0:4], lhsT=lw[:, 0:nt],
                                 rhs=W1Ad_sb[:], start=True, stop=True)
                ad_t = p1.tile([128, 4], bf16, tag="ad_t")
                nc.scalar.activation(ad_t[0:nt, :], ps0[0:nt, 0:4], Act.Copy)
                nc.sync.dma_start(aldb[0][t * 128 : t * 128 + nt, :],
                                  ad_t[0:nt, :])

            # ---------------- phase 1 (layers 2,3): node tables -----------
            def p1_tile(li, t):
                # li in (1, 2): h_aug table for layer li from xT2/xT3
                nt = min(128, vpc - t * 128)
                W_sb = W2aug_sb if li == 1 else W3aug_sb
                aug = 268 if li == 1 else 68
                used = 264 if li == 1 else 66
                row = ROW[li]
                ps1 = ppB.tile([128, 268], f32, tag="psA")
                for k in range(2):
                    lw = p1.tile([128, 128], bf16, tag="lw")
                    nc.sync.dma_start(
                        lw[:, 0:nt],
                        lhsT_srcs[li][k * 128 : (k + 1) * 128,
                                      t * 128 : t * 128 + nt])
                    nc.tensor.matmul(ps1[0:nt, 0:aug], lhsT=lw[:, 0:nt],
                                     rhs=W_sb[k][:], start=(k == 0),
                                     stop=(k == 1))
                hb = p1.tile([128, row], bf16, tag=f"hb{li}")
                nc.vector.tensor_copy(hb[0:nt, 0:used], ps1[0:nt, 0:used])
                if li == 1:
                    # per-head ones columns at 64,129,194,259
                    nc.vector.memset(rd(hb[0:nt, :], 64, [[65, 4]]), 1.0)
                    nc.vector.memset(hb[0:nt, 264:row], 0.0)
                    ad_t = p1.tile([128, 4], bf16, tag="ad_t")
                    nc.scalar.activation(ad_t[0:nt, :], ps1[0:nt, 264:268],
                                         Act.Copy)
                else:
                    nc.vector.memset(hb[0:nt, 64:65], 1.0)
                    nc.vector.memset(hb[0:nt, 66:row], 0.0)
                    ad_t = p1.tile([128, 4], bf16, tag="ad_t")
                    nc.vector.memset(ad_t[0:nt, :], 0.0)
                    nc.scalar.activation(ad_t[0:nt, 0:1], ps1[0:nt, 66:67],
                                         Act.Copy)
                nc.sync.dma_start(tabs_in[li][t * 128 : t * 128 + nt, :],
                                  hb[0:nt, :])
                nc.sync.dma_start(aldb[li][t * 128 : t * 128 + nt, :],
                                  ad_t[0:nt, :])

            def ag_chunk(li, ci):
                r0 = ci * AG_CHUNK
                k0 = ci * ncores * AG_CHUNK
                nc.gpsimd.collective_compute(
                    "AllGather",
                    Alu.bypass,
                    replica_groups=rg,
                    ins=[tabs_in[li][r0 : r0 + AG_CHUNK, :].opt()],
                    outs=[tabs[li][k0 : k0 + ncores * AG_CHUNK, :].opt()],
                )

            # ---------------- phase 2: edge blocks ------------------------
            def p2_block(li, b):
                H = H_[li]
                FH = FH_[li]
                mcol = MCOL[li]
                hg = HG[li]
                als_off = ALS[li]
                rowl = ROWL[li]
                S0, S1 = S0s[b], S1s[b]
                S = S0 + S1
                co = OFF[b]

                sa_sb = ep.tile([128, SMAX * 128], bf16, tag="sa")
                nc.sync.dma_start(sa_sb[:, 0 : S * 128],
                                  sa_in[:, co * 128 : (co + S) * 128])
                saT_sb = ep.tile([128, SMAX * 128], bf16, tag="saT")
                nc.sync.dma_start(saT_sb[:, 0 : S * 128],
                                  saT_in[:, co * 128 : (co + S) * 128])
                aldb_sb = sp.tile([128, 4], bf16, tag="aldb")
                nc.vector.memset(aldb_sb[:], 0.0)
                nc.sync.dma_start(aldb_sb[0:blk, :],
                                  aldb[li][b * blk : (b + 1) * blk, :])

                if li == 0:
                    xe_sb = ep.tile([128, SMAX * 128], bf16, tag="xe")
                    nc.sync.dma_start(xe_sb[:, 0 : S * 128],
                                      xe_in[:, co * 128 : (co + S) * 128])
                    he = ep.tile([128, SMAX, 260], bf16, tag="he")
                    for j in range(S):
                        ph = ppB.tile([128, 268], f32, tag="psA")
                        nc.tensor.matmul(
                            ph[:, 0:260],
                            lhsT=xe_sb[:, j * 128 : (j + 1) * 128],
                            rhs=W1aug_sb[:], start=True, stop=True)
                        nc.scalar.activation(he[:, j, :], ph[:, 0:260],
                                             Act.Copy)
                    src_t = he
                else:
                    eix = sp.tile([128, SMAX * 8], i16, tag="eidx")
                    nc.sync.dma_start(eix[:, 0 : S * 8],
                                      eidx_in[:, co * 8 : (co + S) * 8])
                    g1 = gp.tile([128, SMAX, ROW[li]], bf16, tag="g1")

                    def gath(slot0, nslots, tab_ap):
                        total = nslots * 128
                        for c0 in range(0, total, GATHER_CHUNK):
                            cn = min(GATHER_CHUNK, total - c0)
                            s0 = slot0 + c0 // 128
                            i0 = slot0 * 8 + c0 // 16
                            nc.gpsimd.dma_gather(
                                g1[:, s0 : s0 + cn // 128, :],
                                tab_ap,
                                eix[:, i0 : i0 + cn // 16],
                                cn, cn, ROW[li],
                                queue_num=b % N_SWDGE_Q)

                    gath(0, S0, tabs[li][0:bank, :])
                    gath(S0, S1, tabs[li][bank:n, :])
                    src_t = g1

                # ---- a_dst expansion: dst-local -> per-edge via saT
                alp = ppC.tile([128, SMAX * 4], f32, tag="alp")
                for j in range(S):
                    nc.tensor.matmul(
                        alp[:, j * H : (j + 1) * H],
                        lhsT=saT_sb[:, j * 128 : (j + 1) * 128],
                        rhs=aldb_sb[:, 0:H], start=True, stop=True)
                alf = sp.tile([128, SMAX * 4], f32, tag="alf")
                nc.scalar.activation(alf[:, 0 : S * H], alp[:, 0 : S * H],
                                     Act.Copy)

                # ---- logits -> exp(leaky) = max(exp(0.2 s), exp(s))
                t0 = sp.tile([128, SMAX * 4], f32, tag="t0")
                nc.vector.tensor_tensor(
                    out=t0[:, 0 : S * H],
                    in0=rd(src_t[:], als_off, [[rowl, S], [1, H]]),
                    in1=alf[:, 0 : S * H], op=Alu.add)
                e1 = sp.tile([128, SMAX * 4], f32, tag="e1")
                nc.scalar.activation(e1[:, 0 : S * H], t0[:, 0 : S * H],
                                     Act.Exp, scale=NEG_SLOPE)
                e2 = sp.tile([128, SMAX * 4], f32, tag="e2")
                nc.scalar.activation(e2[:, 0 : S * H], t0[:, 0 : S * H],
                                     Act.Exp)
                exb = sp.tile([128, SMAX * 4], bf16, tag="exb")
                nc.vector.tensor_tensor(
                    out=exb[:, 0 : S * H], in0=e1[:, 0 : S * H],
                    in1=e2[:, 0 : S * H], op=Alu.max)

                # ---- m = h_src * ex (per-head broadcast over C)
                m = ep.tile([128, SMAX, MCOL[li]], bf16, tag="m")
                if li == 0:
                    nc.vector.tensor_tensor(
                        out=rd(m[:], 0, [[256, S], [64, 4], [1, 64]]),
                        in0=rd(src_t[:], 0, [[260, S], [64, 4], [1, 64]]),
                        in1=rd(exb[:], 0, [[4, S], [1, 4], [0, 64]]),
                        op=Alu.mult)
                elif li == 1:
                    nc.vector.tensor_tensor(
                        out=rd(m[:], 0, [[260, S], [65, 4], [1, 65]]),
                        in0=rd(src_t[:], 0, [[384, S], [65, 4], [1, 65]]),
                        in1=rd(exb[:], 0, [[4, S], [1, 4], [0, 65]]),
                        op=Alu.mult)
                else:
                    nc.vector.tensor_tensor(
                        out=rd(m[:], 0, [[65, S], [1, 65]]),
                        in0=rd(src_t[:], 0, [[128, S], [1, 65]]),
                        in1=rd(exb[:], 0, [[1, S], [0, 65]]),
                        op=Alu.mult)

                # ---- scatter-add one-hot matmuls
                ps = ppA.tile([128, 260], f32, tag="ps_sc")
                for j in range(S):
                    nc.tensor.matmul(
                        ps[0:blk, 0 : MCOL[li]],
                        lhsT=sa_sb[:, j * 128 : j * 128 + blk],
                        rhs=m[:, j, :], start=(j == 0), stop=(j == S - 1))
                if li == 0:
                    for j in range(S):
                        nc.tensor.matmul(
                            ps[0:blk, 256:260],
                            lhsT=sa_sb[:, j * 128 : j * 128 + blk],
                            rhs=exb[:, j * 4 : (j + 1) * 4],
                            start=(j == 0), stop=(j == S - 1))

                # ---- epilogue
                rec = sp.tile([128, 4], f32, tag="rec")
                if li == 0:
                    nc.vector.reciprocal(rec[0:blk, :], ps[0:blk, 256:260])
                elif li == 1:
                    nc.vector.reciprocal(rec[0:blk, 0:4],
                                         rd(ps[0:blk, :], 64, [[65, 4]]))
                else:
                    nc.vector.reciprocal(rec[0:blk, 0:1], ps[0:blk, 64:65])
                o = sp.tile([128, 256], f32, tag="o")
                for h in range(H):
                    nc.scalar.activation(
                        o[0:blk, h * 64 : (h + 1) * 64],
                        ps[0:blk, h * hg : h * hg + 64],
                        Act.Copy, scale=rec[0:blk, h : h + 1])
                nc.vector.tensor_tensor(
                    out=o[0:blk, 0:FH], in0=o[0:blk, 0:FH],
                    in1=bias_sb[li][0:blk, 0:FH], op=Alu.add)
                if li < 2:
                    nc.vector.tensor_scalar_max(o[0:blk, 0:FH],
                                                o[0:blk, 0:FH], 0.0)
                if li == 1:
                    xr = sp.tile([128, 256], f32, tag="xr")
                    nc.sync.dma_start(xr[0:blk, :],
                                      x1f[b * blk : (b + 1) * blk, :])
                    nc.vector.tensor_tensor(out=o[0:blk, 0:FH],
                                            in0=o[0:blk, 0:FH],
                                            in1=xr[0:blk, :], op=Alu.add)
                if li == 2:
                    nc.sync.dma_start(out3[b * blk : (b + 1) * blk, :],
                                      o[0:blk, 0:64])
                    return
                if li == 0:
                    nc.sync.dma_start(x1f[b * blk : (b + 1) * blk, :],
                                      o[0:blk, 0:FH])
                ob = sp.tile([128, 256], bf16, tag="ob")
                nc.scalar.activation(ob[0:blk, :], o[0:blk, 0:256], Act.Copy)
                for c2 in range(2):
                    pt = ppT.tile([128, 128], bf16, tag="pt")
                    nc.tensor.transpose(
                        pt[:, 0:blk], ob[0:blk, c2 * 128 : (c2 + 1) * 128],
                        ident_sb[0:blk, 0:blk])
                    st = sp.tile([128, 128], bf16, tag="st")
                    nc.vector.tensor_copy(st[:, 0:blk], pt[:, 0:blk])
                    nc.sync.dma_start(
                        xT_next[li][c2 * 128 : (c2 + 1) * 128,
                                    b * blk : (b + 1) * blk],
                        st[:, 0:blk])

            # ------------- interleaved emission schedule -----------------
            NCHUNK = vpc // AG_CHUNK

            def tiles_ready_after_block(b):
                out = []
                for t in range(NT):
                    nt = min(128, vpc - t * 128)
                    breq = min(nblk - 1, (t * 128 + nt - 1) // blk)
                    if breq == b:
                        out.append(t)
                return out

            def ags_ready_after_tile(t):
                out = []
                for ci in range(NCHUNK):
                    treq = min(NT - 1, (ci * AG_CHUNK + AG_CHUNK - 1) // 128)
                    if treq == t:
                        out.append(ci)
                return out

            for t in range(NT):
                p0_tile(t)
            for li in range(3):
                for b in range(nblk):
                    p2_block(li, b)
                    if li < 2:
                        for t in tiles_ready_after_block(b):
                            p1_tile(li + 1, t)
                            for ci in ags_ready_after_tile(t):
                                ag_chunk(li + 1, ci)
    return nc


# ---------------------------------------------------------------- runner
def _run(per_core, consts, meta, sim=False, trace=False):
    from concourse.bass_utils import run_bass_kernel_spmd

    nc = build_program(meta, consts)
    nc.finalize()
    core_ids = list(range(meta["ncores"]))
    in_maps = [dict(pc) for pc in per_core]
    if sim:
        from concourse.bass_interp import MultiCoreSim

        ms = MultiCoreSim(nc, meta["ncores"])
        for c in core_ids:
            for k, v in in_maps[c].items():
                ms.cores[c].tensor(k)[:] = v
        ms.simulate()
        outs = [np.array(ms.cores[c].tensor("out3")) for c in core_ids]
        return np.concatenate(outs, axis=0), None
    res = run_bass_kernel_spmd(nc, in_maps, core_ids, trace=trace)
    global LAST_EXEC_NS, LAST_RES
    LAST_RES = res
    LAST_EXEC_NS = getattr(res, "exec_time_ns", None)
    outs = [res.results[c]["out3"] for c in core_ids]
    return np.concatenate(outs, axis=0), res


LAST_EXEC_NS = None
LAST_RES = None


def kernel(**inputs):
    x = np.asarray(inputs["x"], np.float32)
    edge_index = np.asarray(inputs["edge_index"])
    cfg = _cfg_full()
    per_core, consts, meta = build_host_data(x, edge_index, inputs, cfg)
    out, _ = _run(
        per_core, consts, meta,
        sim=bool(int(os.environ.get("GAT_SIM", "0"))),
        trace=bool(int(os.environ.get("GAT_TRACE", "0"))),
    )
    return out.astype(np.float32)


# revision 18
# speedup vs baseline: 1.0123x; 1.0123x over previous
"""3-layer GAT (GATConv x3, PyG-style) on Trainium2 across 8 NeuronCores.

v2 design. Destination nodes are 1D-partitioned across 8 cores (6250/core);
edges (self-loops appended) are sorted by dst into per-core blocks of 125
dst nodes, and within a block split into two index banks (int16 gather
indices only reach 32767) and padded to 128-edge "slots".

Per-edge data paths:
  - Layer 1 needs no gather and no collective: the host pre-gathers x[src]
    into per-slot transposed tiles (xe); the device matmuls each slot
    against W1_aug on the PE to get per-edge features directly.
  - Layers 2/3 gather rows of the AllGathered node table (768B / 256B rows)
    with dma_gather; table rows carry per-head [h(64) | 1.0] groups plus
    a_src logits, so the edge-value multiply produces both the weighted
    features and the softmax-denominator columns in one op.
  - a_dst logits are expanded dst->edge with small PE matmuls against
    host-shipped TRANSPOSED one-hot tiles (saT), not a DMA gather.
  - One-hot selection tiles (sa) ship from the host in bf16; the segment
    softmax + scatter-add is PE one-hot matmuls accumulated in PSUM.
  - exp(leaky(s)) = max(exp(0.2 s), exp(s)) via two ACT exps + one DVE max.
  - Epilogue normalization runs on ACT with a per-partition reciprocal
    scale; bias/relu/residual on DVE with contiguous access patterns.

Everything is self-contained: shapes/sharding hardcoded for the nn_GAT
problem (N=50000, E=800000, 128->4x64->4x64->64).
"""

import math
import os
import sys

import numpy as np

sys.path.insert(0, "/opt/trn_rl_repo")

import ml_dtypes

BF16 = ml_dtypes.bfloat16

# ----------------------------------------------------------------- problem
N_NODES = 50000
N_EDGES = 800000
IN_DIM = 128
HID = 64
HEADS = 4
OUT_DIM = 64
NEG_SLOPE = 0.2

N_CORES = 8
BANK = 25000  # int16 gather index bank split (must be <= 32768)
AG_CHUNK = 125  # rows per AllGather chunk (mesh collective regime;
# larger chunks (tried 625 -> 3.8MB, RDH regime) crash the NEFF)
GATHER_CHUNK = 1024  # max indices per dma_gather call
N_SWDGE_Q = 1  # SWDGE queues used round-robin for gathers
DMA_SCRATCH = 16384  # descriptor-ring carveout bytes/partition

ROW = {1: 384, 2: 128}  # gathered table row sizes (bf16 elems)


def _cfg_full():
    return dict(
        n=N_NODES,
        ncores=N_CORES,
        vpc=N_NODES // N_CORES,
        blk=125,
        bank=BANK,
    )


# ------------------------------------------------------------- host prep
def _wrap_idx(idx):
    """[n] int array -> [128, n//16] int16 in the SWDGE wrapped layout
    (position i lives at partition i%16, column i//16; replicated x8)."""
    n = idx.shape[0]
    assert n % 16 == 0
    w = np.asarray(idx, np.int16).reshape(n // 16, 16).T  # [16, n//16]
    return np.tile(w, (8, 1))  # [128, n//16]


def _remap(g, vpc, ncores):
    """Node id -> row in the chunk-interleaved AllGather output table."""
    rank, within = g // vpc, g % vpc
    chunk, i = within // AG_CHUNK, within % AG_CHUNK
    return chunk * (ncores * AG_CHUNK) + rank * AG_CHUNK + i


def build_host_data(x, edge_index, Ws, cfg):
    n = cfg["n"]
    ncores = cfg["ncores"]
    vpc = cfg["vpc"]
    blk = cfg["blk"]
    bank = cfg["bank"]
    nblk = vpc // blk
    assert nblk * blk == vpc and vpc * ncores == n

    src = np.concatenate([np.asarray(edge_index[0], np.int64), np.arange(n)])
    dst = np.concatenate([np.asarray(edge_index[1], np.int64), np.arange(n)])
    order = np.argsort(dst, kind="stable")
    src, dst = src[order], dst[order]
    srcR = _remap(src, vpc, ncores)

    gblk = dst // blk
    nb_all = ncores * nblk
    bstart = np.searchsorted(gblk, np.arange(nb_all))
    bend = np.searchsorted(gblk, np.arange(nb_all), side="right")

    # per-block-position slot counts, maxed over cores (SPMD program)
    S0s, S1s = [], []
    for cb in range(nblk):
        m0 = m1 = 1
        for c in range(ncores):
            b = c * nblk + cb
            s = srcR[bstart[b] : bend[b]]
            n0 = int((s < bank).sum())
            n1 = int(len(s) - n0)
            m0 = max(m0, math.ceil(max(n0, 1) / 128))
            m1 = max(m1, math.ceil(max(n1, 1) / 128))
        S0s.append(m0)
        S1s.append(m1)
    Sb = [a + b for a, b in zip(S0s, S1s)]
    OFF = np.concatenate([[0], np.cumsum(Sb)]).astype(int)
    TOT = int(OFF[-1])
    SMAX = max(Sb)

    xT_full = np.ascontiguousarray(np.asarray(x, np.float32).T).astype(BF16)

    per_core = []
    for c in range(ncores):
        eidx = np.zeros((128, TOT * 8), np.int16)
        sa = np.zeros((128, TOT * 128), BF16)
        saT = np.zeros((128, TOT * 128), BF16)
        xe = np.zeros((128, TOT * 128), BF16)
        for cb in range(nblk):
            b = c * nblk + cb
            lo, hi = bstart[b], bend[b]
            sR, sO = srcR[lo:hi], src[lo:hi]
            d = (dst[lo:hi] - b * blk).astype(np.int64)
            in0 = sR < bank
            co = int(OFF[cb])
            S0 = S0s[cb]
            for half, (sRh, sOh, dh, soff, scnt) in enumerate(
                [
                    (sR[in0], sO[in0], d[in0], 0, S0),
                    (sR[~in0] - bank, sO[~in0], d[~in0], S0, S1s[cb]),
                ]
            ):
                k = np.arange(len(sRh))
                part = k % 128
                cols = (co + soff + k // 128) * 128
                sa[part, cols + dh] = 1.0
                saT[dh, cols + part] = 1.0
                xe[:, cols + part] = xT_full[:, sOh]
                idx = np.zeros(scnt * 128, np.int16)
                idx[: len(sRh)] = sRh
                eidx[:, (co + soff) * 8 : (co + soff + scnt) * 8] = _wrap_idx(idx)
        xT = np.ascontiguousarray(xT_full[:, c * vpc : (c + 1) * vpc])
        per_core.append(dict(xT=xT, eidx=eidx, sa=sa, saT=saT, xe=xe))

    # ---- shared constants
    def headfold(W, a):
        # [fin, H*C] x [H, C] -> [fin, H] per-head logit weights
        H, C = a.shape
        return np.stack(
            [W[:, h * C : (h + 1) * C] @ a[h] for h in range(H)], axis=1
        )

    W1 = np.asarray(Ws["W1"], np.float32)
    W2 = np.asarray(Ws["W2"], np.float32)
    W3 = np.asarray(Ws["W3"], np.float32)
    As1 = headfold(W1, np.asarray(Ws["as1"], np.float32))
    Ad1 = headfold(W1, np.asarray(Ws["ad1"], np.float32))
    As2 = headfold(W2, np.asarray(Ws["as2"], np.float32))
    Ad2 = headfold(W2, np.asarray(Ws["ad2"], np.float32))
    As3 = headfold(W3, np.asarray(Ws["as3"], np.float32))
    Ad3 = headfold(W3, np.asarray(Ws["ad3"], np.float32))

    W1aug = np.concatenate([W1, As1], axis=1)  # [128, 260]
    W2aug = np.zeros((256, 268), np.float32)
    for h in range(4):
        W2aug[:, h * 65 : h * 65 + 64] = W2[:, h * 64 : (h + 1) * 64]
    W2aug[:, 260:264] = As2
    W2aug[:, 264:268] = Ad2
    W3aug = np.zeros((256, 68), np.float32)
    W3aug[:, 0:64] = W3
    W3aug[:, 65:66] = As3
    W3aug[:, 66:67] = Ad3

    consts = dict(
        W1aug=W1aug.astype(BF16),
        W1Ad=Ad1.astype(BF16),
        W2aug=W2aug.astype(BF16),
        W3aug=W3aug.astype(BF16),
        b1=np.tile(np.asarray(Ws["b1"], np.float32)[None, :], (128, 1)),
        b2=np.tile(np.asarray(Ws["b2"], np.float32)[None, :], (128, 1)),
        b3=np.tile(np.asarray(Ws["b3"], np.float32)[None, :], (128, 1)),
        ident=np.eye(128, dtype=np.float32).astype(BF16),
    )
    meta = dict(S0s=S0s, S1s=S1s, OFF=OFF.tolist(), TOT=TOT, SMAX=SMAX,
                nblk=nblk, **cfg)
    return per_core, consts, meta


# ------------------------------------------------------------ device build
def build_program(meta, consts):
    import concourse.bass as bass
    import concourse.mybir as mybir
    import concourse.tile as tile
    from concourse import bacc

    f32 = mybir.dt.float32
    bf16 = mybir.dt.bfloat16
    i16 = mybir.dt.int16
    Alu = mybir.AluOpType
    Act = mybir.ActivationFunctionType

    n = meta["n"]
    ncores = meta["ncores"]
    vpc = meta["vpc"]
    blk = meta["blk"]
    nblk = meta["nblk"]
    bank = meta["bank"]
    S0s, S1s, OFF = meta["S0s"], meta["S1s"], meta["OFF"]
    TOT, SMAX = meta["TOT"], meta["SMAX"]
    NT = math.ceil(vpc / 128)

    # per-layer static dims
    H_ = {0: 4, 1: 4, 2: 1}
    FH_ = {0: 256, 1: 256, 2: 64}
    MCOL = {0: 256, 1: 260, 2: 65}  # scatter rhs width
    HG = {0: 64, 1: 65, 2: 65}  # per-head stride in ps_sc
    ALS = {0: 256, 1: 260, 2: 65}  # a_src column offset in edge rows
    ROWL = {0: 260, 1: ROW[1], 2: ROW[2]}  # edge-row stride

    nc = bacc.Bacc(trn_type="TRN2", num_devices=ncores,
                   num_swdge_queues=N_SWDGE_Q,
                   dynamic_dma_scratch_size=DMA_SCRATCH)
    rg = [list(range(ncores))]

    # ---------------- I/O ----------------
    xT_in = nc.dram_tensor("xT", [128, vpc], bf16, kind="ExternalInput")
    eidx_in = nc.dram_tensor("eidx", [128, TOT * 8], i16, kind="ExternalInput")
    sa_in = nc.dram_tensor("sa", [128, TOT * 128], bf16, kind="ExternalInput")
    saT_in = nc.dram_tensor("saT", [128, TOT * 128], bf16, kind="ExternalInput")
    xe_in = nc.dram_tensor("xe", [128, TOT * 128], bf16, kind="ExternalInput")
    out3 = nc.dram_tensor("out3", [vpc, OUT_DIM], f32, kind="ExternalOutput")

    W1aug_t = nc.inline_tensor(consts["W1aug"], "W1aug")
    W1Ad_t = nc.inline_tensor(consts["W1Ad"], "W1Ad")
    W2aug_t = nc.inline_tensor(consts["W2aug"], "W2aug")
    W3aug_t = nc.inline_tensor(consts["W3aug"], "W3aug")
    b1_t = nc.inline_tensor(consts["b1"], "b1c")
    b2_t = nc.inline_tensor(consts["b2"], "b2c")
    b3_t = nc.inline_tensor(consts["b3"], "b3c")
    ident_t = nc.inline_tensor(consts["ident"], "identc")

    # internal DRAM
    tabs_in = {li: nc.dram_tensor(f"tab{li}_in", [vpc, ROW[li]], bf16)
               for li in (1, 2)}
    tabs = {li: nc.dram_tensor(f"tab{li}", [n, ROW[li]], bf16,
                               addr_space="Shared") for li in (1, 2)}
    aldb = {li: nc.dram_tensor(f"aldb{li}", [vpc, 4], bf16) for li in (0, 1, 2)}
    x1f = nc.dram_tensor("x1f", [vpc, 256], f32)
    xT2 = nc.dram_tensor("xT2", [256, vpc], bf16)
    xT3 = nc.dram_tensor("xT3", [256, vpc], bf16)
    xT_next = {0: xT2, 1: xT3}
    lhsT_srcs = {1: xT2, 2: xT3}

    AP = bass.AP

    def rd(ap, offset_elems, dims):
        return AP(ap.tensor, ap.offset + offset_elems,
                  [list(ap.ap[0])] + [list(d) for d in dims])

    with tile.TileContext(nc) as tc:
        with (
            tc.tile_pool(name="const", bufs=1) as cpool,
            tc.tile_pool(name="p1", bufs=3) as p1,
            tc.tile_pool(name="g", bufs=3) as gp,
            tc.tile_pool(name="e", bufs=2) as ep,
            tc.tile_pool(name="small", bufs=4) as sp,
            tc.tile_pool(name="psA", bufs=2, space="PSUM") as ppA,
            tc.tile_pool(name="psB", bufs=2, space="PSUM") as ppB,
            tc.tile_pool(name="psC", bufs=2, space="PSUM") as ppC,
            tc.tile_pool(name="psumT", bufs=2, space="PSUM") as ppT,
        ):
            ident_sb = cpool.tile([128, 128], bf16, tag="ident")
            nc.sync.dma_start(ident_sb[:], ident_t[:])
            bias_sb = []
            for li, bt in enumerate([b1_t, b2_t, b3_t]):
                b_sb = cpool.tile([128, bt.shape[1]], f32, tag=f"bias{li}")
                nc.sync.dma_start(b_sb[:], bt[:])
                bias_sb.append(b_sb)
            W1aug_sb = cpool.tile([128, 260], bf16, tag="w1aug")
            nc.sync.dma_start(W1aug_sb[:], W1aug_t[:])
            W1Ad_sb = cpool.tile([128, 4], bf16, tag="w1ad")
            nc.sync.dma_start(W1Ad_sb[:], W1Ad_t[:])
            W2aug_sb = []
            for k in range(2):
                w_sb = cpool.tile([128, 268], bf16, tag=f"w2aug{k}")
                nc.sync.dma_start(w_sb[:], W2aug_t[k * 128 : (k + 1) * 128, :])
                W2aug_sb.append(w_sb)
            W3aug_sb = []
            for k in range(2):
                w_sb = cpool.tile([128, 68], bf16, tag=f"w3aug{k}")
                nc.sync.dma_start(w_sb[:], W3aug_t[k * 128 : (k + 1) * 128, :])
                W3aug_sb.append(w_sb)

            # ---------------- phase 0: aldb[0] = x @ (W1 A_d) -------------
            def p0_tile(t):
                nt = min(128, vpc - t * 128)
                lw = p1.tile([128, 128], bf16, tag="lw")
                nc.sync.dma_start(lw[:, 0:nt],
                                  xT_in[:, t * 128 : t * 128 + nt])
                ps0 = ppB.tile([128, 268], f32, tag="psA")
                nc.tensor.matmul(ps0[0:nt, 0:4], lhsT=lw[:, 0:nt],
                                 rhs=W1Ad_sb[:], start=True, stop=True)
                ad_t = p1.tile([128, 4], bf16, tag="ad_t")
                nc.scalar.activation(ad_t[0:nt, :], ps0[0:nt, 0:4], Act.Copy)
                nc.sync.dma_start(aldb[0][t * 128 : t * 128 + nt, :],
                                  ad_t[0:nt, :])

            # ---------------- phase 1 (layers 2,3): node tables -----------
            def p1_tile(li, t):
                # li in (1, 2): h_aug table for layer li from xT2/xT3
                nt = min(128, vpc - t * 128)
                W_sb = W2aug_sb if li == 1 else W3aug_sb
                aug = 268 if li == 1 else 68
                used = 264 if li == 1 else 66
                row = ROW[li]
                ps1 = ppB.tile([128, 268], f32, tag="psA")
                for k in range(2):
                    lw = p1.tile([128, 128], bf16, tag="lw")
                    nc.sync.dma_start(
                        lw[:, 0:nt],
                        lhsT_srcs[li][k * 128 : (k + 1) * 128,
                                      t * 128 : t * 128 + nt])
                    nc.tensor.matmul(ps1[0:nt, 0:aug], lhsT=lw[:, 0:nt],
                                     rhs=W_sb[k][:], start=(k == 0),
                                     stop=(k == 1))
                hb = p1.tile([128, row], bf16, tag=f"hb{li}")
                nc.vector.tensor_copy(hb[0:nt, 0:used], ps1[0:nt, 0:used])
                if li == 1:
                    # per-head ones columns at 64,129,194,259
                    nc.vector.memset(rd(hb[0:nt, :], 64, [[65, 4]]), 1.0)
                    nc.vector.memset(hb[0:nt, 264:row], 0.0)
                    ad_t = p1.tile([128, 4], bf16, tag="ad_t")
                    nc.scalar.activation(ad_t[0:nt, :], ps1[0:nt, 264:268],
                                         Act.Copy)
                else:
                    nc.vector.memset(hb[0:nt, 64:65], 1.0)
                    nc.vector.memset(hb[0:nt, 66:row], 0.0)
                    ad_t = p1.tile([128, 4], bf16, tag="ad_t")
                    nc.vector.memset(ad_t[0:nt, :], 0.0)
                    nc.scalar.activation(ad_t[0:nt, 0:1], ps1[0:nt, 66:67],
                                         Act.Copy)
                nc.sync.dma_start(tabs_in[li][t * 128 : t * 128 + nt, :],
                                  hb[0:nt, :])
                nc.sync.dma_start(aldb[li][t * 128 : t * 128 + nt, :],
                                  ad_t[0:nt, :])

            def ag_chunk(li, ci):
                r0 = ci * AG_CHUNK
                k0 = ci * ncores * AG_CHUNK
                nc.gpsimd.collective_compute(
                    "AllGather",
                    Alu.bypass,
                    replica_groups=rg,
                    ins=[tabs_in[li][r0 : r0 + AG_CHUNK, :].opt()],
                    outs=[tabs[li][k0 : k0 + ncores * AG_CHUNK, :].opt()],
                )

            # ---------------- phase 2: edge blocks ------------------------
            def p2_block(li, b):
                H = H_[li]
                FH = FH_[li]
                mcol = MCOL[li]
                hg = HG[li]
                als_off = ALS[li]
                rowl = ROWL[li]
                S0, S1 = S0s[b], S1s[b]
                S = S0 + S1
                co = OFF[b]

                sa_sb = ep.tile([128, SMAX * 128], bf16, tag="sa")
                nc.sync.dma_start(sa_sb[:, 0 : S * 128],
                                  sa_in[:, co * 128 : (co + S) * 128])
                saT_sb = ep.tile([128, SMAX * 128], bf16, tag="saT")
                nc.sync.dma_start(saT_sb[:, 0 : S * 128],
                                  saT_in[:, co * 128 : (co + S) * 128])
                aldb_sb = sp.tile([128, 4], bf16, tag="aldb")
                nc.vector.memset(aldb_sb[:], 0.0)
                nc.sync.dma_start(aldb_sb[0:blk, :],
                                  aldb[li][b * blk : (b + 1) * blk, :])

                if li == 0:
                    xe_sb = ep.tile([128, SMAX * 128], bf16, tag="xe")
                    nc.sync.dma_start(xe_sb[:, 0 : S * 128],
                                      xe_in[:, co * 128 : (co + S) * 128])
                    he = ep.tile([128, SMAX, 260], bf16, tag="he")
                    for j in range(S):
                        ph = ppB.tile([128, 268], f32, tag="psA")
                        nc.tensor.matmul(
                            ph[:, 0:260],
                            lhsT=xe_sb[:, j * 128 : (j + 1) * 128],
                            rhs=W1aug_sb[:], start=True, stop=True)
                        nc.scalar.activation(he[:, j, :], ph[:, 0:260],
                                             Act.Copy)
                    src_t = he
                else:
                    eix = sp.tile([128, SMAX * 8], i16, tag="eidx")
                    nc.sync.dma_start(eix[:, 0 : S * 8],
                                      eidx_in[:, co * 8 : (co + S) * 8])
                    g1 = gp.tile([128, SMAX, ROW[li]], bf16, tag="g1")

                    def gath(slot0, nslots, tab_ap):
                        total = nslots * 128
                        for c0 in range(0, total, GATHER_CHUNK):
                            cn = min(GATHER_CHUNK, total - c0)
                            s0 = slot0 + c0 // 128
                            i0 = slot0 * 8 + c0 // 16
                            nc.gpsimd.dma_gather(
                                g1[:, s0 : s0 + cn // 128, :],
                                tab_ap,
                                eix[:, i0 : i0 + cn // 16],
                                cn, cn, ROW[li],
                                queue_num=b % N_SWDGE_Q)

                    gath(0, S0, tabs[li][0:bank, :])
                    gath(S0, S1, tabs[li][bank:n, :])
                    src_t = g1

                # ---- a_dst expansion: dst-local -> per-edge via saT
                alp = ppC.tile([128, SMAX * 4], f32, tag="alp")
                for j in range(S):
                    nc.tensor.matmul(
                        alp[:, j * H : (j + 1) * H],
                        lhsT=saT_sb[:, j * 128 : (j + 1) * 128],
                        rhs=aldb_sb[:, 0:H], start=True, stop=True)
                alf = sp.tile([128, SMAX * 4], f32, tag="alf")
                nc.scalar.activation(alf[:, 0 : S * H], alp[:, 0 : S * H],
                                     Act.Copy)

                # ---- logits -> exp(leaky) = max(exp(0.2 s), exp(s))
                t0 = sp.tile([128, SMAX * 4], f32, tag="t0")
                nc.vector.tensor_tensor(
                    out=t0[:, 0 : S * H],
                    in0=rd(src_t[:], als_off, [[rowl, S], [1, H]]),
                    in1=alf[:, 0 : S * H], op=Alu.add)
                e1 = sp.tile([128, SMAX * 4], f32, tag="e1")
                nc.scalar.activation(e1[:, 0 : S * H], t0[:, 0 : S * H],
                                     Act.Exp, scale=NEG_SLOPE)
                e2 = sp.tile([128, SMAX * 4], f32, tag="e2")
                nc.scalar.activation(e2[:, 0 : S * H], t0[:, 0 : S * H],
                                     Act.Exp)
                exb = sp.tile([128, SMAX * 4], bf16, tag="exb")
                nc.vector.tensor_tensor(
                    out=exb[:, 0 : S * H], in0=e1[:, 0 : S * H],
                    in1=e2[:, 0 : S * H], op=Alu.max)

                # ---- m = h_src * ex (per-head broadcast over C)
                m = ep.tile([128, SMAX, MCOL[li]], bf16, tag="m")
                if li == 0:
                    nc.vector.tensor_tensor(
                        out=rd(m[:], 0, [[256, S], [64, 4], [1, 64]]),
                        in0=rd(src_t[:], 0, [[260, S], [64, 4], [1, 64]]),
                        in1=rd(exb[:], 0, [[4, S], [1, 4], [0, 64]]),
                        op=Alu.mult)
                elif li == 1:
                    nc.vector.tensor_tensor(
                        out=rd(m[:], 0, [[260, S], [65, 4], [1, 65]]),
                        in0=rd(src_t[:], 0, [[384, S], [65, 4], [1, 65]]),
                        in1=rd(exb[:], 0, [[4, S], [1, 4], [0, 65]]),
                        op=Alu.mult)
                else:
                    nc.vector.tensor_tensor(
                        out=rd(m[:], 0, [[65, S], [1, 65]]),
                        in0=rd(src_t[:], 0, [[128, S], [1, 65]]),
                        in1=rd(exb[:], 0, [[1, S], [0, 65]]),
                        op=Alu.mult)

                # ---- scatter-add one-hot matmuls
                ps = ppA.tile([128, 260], f32, tag="ps_sc")
                for j in range(S):
                    nc.tensor.matmul(
                        ps[0:blk, 0 : MCOL[li]],
                        lhsT=sa_sb[:, j * 128 : j * 128 + blk],
                        rhs=m[:, j, :], start=(j == 0), stop=(j == S - 1))
                if li == 0:
                    for j in range(S):
                        nc.tensor.matmul(
                            ps[0:blk, 256:260],
                            lhsT=sa_sb[:, j * 128 : j * 128 + blk],
                            rhs=exb[:, j * 4 : (j + 1) * 4],
                            start=(j == 0), stop=(j == S - 1))

                # ---- epilogue
                rec = sp.tile([128, 4], f32, tag="rec")
                if li == 0:
                    nc.vector.reciprocal(rec[0:blk, :], ps[0:blk, 256:260])
                elif li == 1:
                    nc.vector.reciprocal(rec[0:blk, 0:4],
                                         rd(ps[0:blk, :], 64, [[65, 4]]))
                else:
                    nc.vector.reciprocal(rec[0:blk, 0:1], ps[0:blk, 64:65])
                o = sp.tile([128, 256], f32, tag="o")
                for h in range(H):
                    nc.scalar.activation(
                        o[0:blk, h * 64 : (h + 1) * 64],
                        ps[0:blk, h * hg : h * hg + 64],
                        Act.Copy, scale=rec[0:blk, h : h + 1])
                nc.vector.tensor_tensor(
                    out=o[0:blk, 0:FH], in0=o[0:blk, 0:FH],
                    in1=bias_sb[li][0:blk, 0:FH], op=Alu.add)
                if li < 2:
                    nc.vector.tensor_scalar_max(o[0:blk, 0:FH],
                                                o[0:blk, 0:FH], 0.0)
                if li == 1:
                    xr = sp.tile([128, 256], f32, tag="xr")
                    nc.sync.dma_start(xr[0:blk, :],
                                      x1f[b * blk : (b + 1) * blk, :])
                    nc.vector.tensor_tensor(out=o[0:blk, 0:FH],
                                            in0=o[0:blk, 0:FH],
                                            in1=xr[0:blk, :], op=Alu.add)
                if li == 2:
                    nc.sync.dma_start(out3[b * blk : (b + 1) * blk, :],
                                      o[0:blk, 0:64])
                    return
                if li == 0:
                    nc.sync.dma_start(x1f[b * blk : (b + 1) * blk, :],
                                      o[0:blk, 0:FH])
                ob = sp.tile([128, 256], bf16, tag="ob")
                nc.scalar.activation(ob[0:blk, :], o[0:blk, 0:256], Act.Copy)
                for c2 in range(2):
                    pt = ppT.tile([128, 128], bf16, tag="pt")
                    nc.tensor.transpose(
                        pt[:, 0:blk], ob[0:blk, c2 * 128 : (c2 + 1) * 128],
                        ident_sb[0:blk, 0:blk])
                    st = sp.tile([128, 128], bf16, tag="st")
                    nc.vector.tensor_copy(st[:, 0:blk], pt[:, 0:blk])
                    nc.sync.dma_start(
                        xT_next[li][c2 * 128 : (c2 + 1) * 128,
                                    b * blk : (b + 1) * blk],
                        st[:, 0:blk])

            # ------------- interleaved emission schedule -----------------
            NCHUNK = vpc // AG_CHUNK

            def tiles_ready_after_block(b):
                out = []
                for t in range(NT):
                    nt = min(128, vpc - t * 128)
                    breq = min(nblk - 1, (t * 128 + nt - 1) // blk)
                    if breq == b:
                        out.append(t)
                return out

            def ags_ready_after_tile(t):
                out = []
                for ci in range(NCHUNK):
                    treq = min(NT - 1, (ci * AG_CHUNK + AG_CHUNK - 1) // 128)
                    if treq == t:
                        out.append(ci)
                return out

            for t in range(NT):
                p0_tile(t)
            for li in range(3):
                for b in range(nblk):
                    p2_block(li, b)
                    if li < 2:
                        for t in tiles_ready_after_block(b):
                            p1_tile(li + 1, t)
                            for ci in ags_ready_after_tile(t):
                                ag_chunk(li + 1, ci)
    return nc


# ---------------------------------------------------------------- runner
def _run(per_core, consts, meta, sim=False, trace=False):
    from concourse.bass_utils import run_bass_kernel_spmd

    nc = build_program(meta, consts)
    nc.finalize()
    core_ids = list(range(meta["ncores"]))
    in_maps = [dict(pc) for pc in per_core]
    if sim:
        from concourse.bass_interp import MultiCoreSim

        ms = MultiCoreSim(nc, meta["ncores"])
        for c in core_ids:
            for k, v in in_maps[c].items():
                ms.cores[c].tensor(k)[:] = v
        ms.simulate()
        outs = [np.array(ms.cores[c].tensor("out3")) for c in core_ids]
        return np.concatenate(outs, axis=0), None
    res = run_bass_kernel_spmd(nc, in_maps, core_ids, trace=trace)
    global LAST_EXEC_NS, LAST_RES
    LAST_RES = res
    LAST_EXEC_NS = getattr(res, "exec_time_ns", None)
    outs = [res.results[c]["out3"] for c in core_ids]
    return np.concatenate(outs, axis=0), res


LAST_EXEC_NS = None
LAST_RES = None


def kernel(**inputs):
    x = np.asarray(inputs["x"], np.float32)
    edge_index = np.asarray(inputs["edge_index"])
    cfg = _cfg_full()
    per_core, consts, meta = build_host_data(x, edge_index, inputs, cfg)
    out, _ = _run(
        per_core, consts, meta,
        sim=bool(int(os.environ.get("GAT_SIM", "0"))),
        trace=bool(int(os.environ.get("GAT_TRACE", "0"))),
    )
    return out.astype(np.float32)


# revision 19
# speedup vs baseline: 1.0144x; 1.0021x over previous
"""3-layer GAT (GATConv x3, PyG-style) on Trainium2 across 8 NeuronCores.

v2 design. Destination nodes are 1D-partitioned across 8 cores (6250/core);
edges (self-loops appended) are sorted by dst into per-core blocks of 125
dst nodes, and within a block split into two index banks (int16 gather
indices only reach 32767) and padded to 128-edge "slots".

Per-edge data paths:
  - Layer 1 needs no gather and no collective: the host pre-gathers x[src]
    into per-slot transposed tiles (xe); the device matmuls each slot
    against W1_aug on the PE to get per-edge features directly.
  - Layers 2/3 gather rows of the AllGathered node table (768B / 256B rows)
    with dma_gather; table rows carry per-head [h(64) | 1.0] groups plus
    a_src logits, so the edge-value multiply produces both the weighted
    features and the softmax-denominator columns in one op.
  - a_dst logits are expanded dst->edge with small PE matmuls against
    host-shipped TRANSPOSED one-hot tiles (saT), not a DMA gather.
  - One-hot selection tiles (sa) ship from the host in bf16; the segment
    softmax + scatter-add is PE one-hot matmuls accumulated in PSUM.
  - exp(leaky(s)) = max(exp(0.2 s), exp(s)) via two ACT exps + one DVE max.
  - Epilogue normalization runs on ACT with a per-partition reciprocal
    scale; bias/relu/residual on DVE with contiguous access patterns.

Everything is self-contained: shapes/sharding hardcoded for the nn_GAT
problem (N=50000, E=800000, 128->4x64->4x64->64).
"""

import math
import os
import sys

import numpy as np

sys.path.insert(0, "/opt/trn_rl_repo")

import ml_dtypes

BF16 = ml_dtypes.bfloat16

# ----------------------------------------------------------------- problem
N_NODES = 50000
N_EDGES = 800000
IN_DIM = 128
HID = 64
HEADS = 4
OUT_DIM = 64
NEG_SLOPE = 0.2

N_CORES = 8
BANK = 25000  # int16 gather index bank split (must be <= 32768)
AG_CHUNK = 125  # rows per AllGather chunk (mesh collective regime;
# larger chunks (tried 625 -> 3.8MB, RDH regime) crash the NEFF)
GATHER_CHUNK = 1024  # max indices per dma_gather call
N_SWDGE_Q = 1  # SWDGE queues used round-robin for gathers
DMA_SCRATCH = 16384  # descriptor-ring carveout bytes/partition

ROW = {1: 384, 2: 128}  # gathered table row sizes (bf16 elems)


def _cfg_full():
    return dict(
        n=N_NODES,
        ncores=N_CORES,
        vpc=N_NODES // N_CORES,
        blk=125,
        bank=BANK,
    )


# ------------------------------------------------------------- host prep
def _wrap_idx(idx):
    """[n] int array -> [128, n//16] int16 in the SWDGE wrapped layout
    (position i lives at partition i%16, column i//16; replicated x8)."""
    n = idx.shape[0]
    assert n % 16 == 0
    w = np.asarray(idx, np.int16).reshape(n // 16, 16).T  # [16, n//16]
    return np.tile(w, (8, 1))  # [128, n//16]


def _remap(g, vpc, ncores):
    """Node id -> row in the chunk-interleaved AllGather output table."""
    rank, within = g // vpc, g % vpc
    chunk, i = within // AG_CHUNK, within % AG_CHUNK
    return chunk * (ncores * AG_CHUNK) + rank * AG_CHUNK + i


def build_host_data(x, edge_index, Ws, cfg):
    n = cfg["n"]
    ncores = cfg["ncores"]
    vpc = cfg["vpc"]
    blk = cfg["blk"]
    bank = cfg["bank"]
    nblk = vpc // blk
    assert nblk * blk == vpc and vpc * ncores == n

    src = np.concatenate([np.asarray(edge_index[0], np.int64), np.arange(n)])
    dst = np.concatenate([np.asarray(edge_index[1], np.int64), np.arange(n)])
    order = np.argsort(dst, kind="stable")
    src, dst = src[order], dst[order]
    srcR = _remap(src, vpc, ncores)

    gblk = dst // blk
    nb_all = ncores * nblk
    bstart = np.searchsorted(gblk, np.arange(nb_all))
    bend = np.searchsorted(gblk, np.arange(nb_all), side="right")

    # per-block-position slot counts, maxed over cores (SPMD program)
    S0s, S1s = [], []
    for cb in range(nblk):
        m0 = m1 = 1
        for c in range(ncores):
            b = c * nblk + cb
            s = srcR[bstart[b] : bend[b]]
            n0 = int((s < bank).sum())
            n1 = int(len(s) - n0)
            m0 = max(m0, math.ceil(max(n0, 1) / 128))
            m1 = max(m1, math.ceil(max(n1, 1) / 128))
        S0s.append(m0)
        S1s.append(m1)
    Sb = [a + b for a, b in zip(S0s, S1s)]
    OFF = np.concatenate([[0], np.cumsum(Sb)]).astype(int)
    TOT = int(OFF[-1])
    SMAX = max(Sb)

    xT_full = np.ascontiguousarray(np.asarray(x, np.float32).T).astype(BF16)

    per_core = []
    for c in range(ncores):
        eidx = np.zeros((128, TOT * 8), np.int16)
        sa = np.zeros((128, TOT * 128), BF16)
        saT = np.zeros((128, TOT * 128), BF16)
        xe = np.zeros((128, TOT * 128), BF16)
        for cb in range(nblk):
            b = c * nblk + cb
            lo, hi = bstart[b], bend[b]
            sR, sO = srcR[lo:hi], src[lo:hi]
            d = (dst[lo:hi] - b * blk).astype(np.int64)
            in0 = sR < bank
            co = int(OFF[cb])
            S0 = S0s[cb]
            for half, (sRh, sOh, dh, soff, scnt) in enumerate(
                [
                    (sR[in0], sO[in0], d[in0], 0, S0),
                    (sR[~in0] - bank, sO[~in0], d[~in0], S0, S1s[cb]),
                ]
            ):
                k = np.arange(len(sRh))
                part = k % 128
                cols = (co + soff + k // 128) * 128
                sa[part, cols + dh] = 1.0
                saT[dh, cols + part] = 1.0
                xe[:, cols + part] = xT_full[:, sOh]
                idx = np.zeros(scnt * 128, np.int16)
                idx[: len(sRh)] = sRh
                eidx[:, (co + soff) * 8 : (co + soff + scnt) * 8] = _wrap_idx(idx)
        xT = np.ascontiguousarray(xT_full[:, c * vpc : (c + 1) * vpc])
        per_core.append(dict(xT=xT, eidx=eidx, sa=sa, saT=saT, xe=xe))

    # ---- shared constants
    def headfold(W, a):
        # [fin, H*C] x [H, C] -> [fin, H] per-head logit weights
        H, C = a.shape
        return np.stack(
            [W[:, h * C : (h + 1) * C] @ a[h] for h in range(H)], axis=1
        )

    W1 = np.asarray(Ws["W1"], np.float32)
    W2 = np.asarray(Ws["W2"], np.float32)
    W3 = np.asarray(Ws["W3"], np.float32)
    As1 = headfold(W1, np.asarray(Ws["as1"], np.float32))
    Ad1 = headfold(W1, np.asarray(Ws["ad1"], np.float32))
    As2 = headfold(W2, np.asarray(Ws["as2"], np.float32))
    Ad2 = headfold(W2, np.asarray(Ws["ad2"], np.float32))
    As3 = headfold(W3, np.asarray(Ws["as3"], np.float32))
    Ad3 = headfold(W3, np.asarray(Ws["ad3"], np.float32))

    W1aug = np.concatenate([W1, As1], axis=1)  # [128, 260]
    W2aug = np.zeros((256, 268), np.float32)
    for h in range(4):
        W2aug[:, h * 65 : h * 65 + 64] = W2[:, h * 64 : (h + 1) * 64]
    W2aug[:, 260:264] = As2
    W2aug[:, 264:268] = Ad2
    W3aug = np.zeros((256, 68), np.float32)
    W3aug[:, 0:64] = W3
    W3aug[:, 65:66] = As3
    W3aug[:, 66:67] = Ad3

    consts = dict(
        W1aug=W1aug.astype(BF16),
        W1Ad=Ad1.astype(BF16),
        W2aug=W2aug.astype(BF16),
        W3aug=W3aug.astype(BF16),
        b1=np.tile(np.asarray(Ws["b1"], np.float32)[None, :], (128, 1)),
        b2=np.tile(np.asarray(Ws["b2"], np.float32)[None, :], (128, 1)),
        b3=np.tile(np.asarray(Ws["b3"], np.float32)[None, :], (128, 1)),
        ident=np.eye(128, dtype=np.float32).astype(BF16),
    )
    meta = dict(S0s=S0s, S1s=S1s, OFF=OFF.tolist(), TOT=TOT, SMAX=SMAX,
                nblk=nblk, **cfg)
    return per_core, consts, meta


# ------------------------------------------------------------ device build
def build_program(meta, consts):
    import concourse.bass as bass
    import concourse.mybir as mybir
    import concourse.tile as tile
    from concourse import bacc

    f32 = mybir.dt.float32
    bf16 = mybir.dt.bfloat16
    i16 = mybir.dt.int16
    Alu = mybir.AluOpType
    Act = mybir.ActivationFunctionType

    n = meta["n"]
    ncores = meta["ncores"]
    vpc = meta["vpc"]
    blk = meta["blk"]
    nblk = meta["nblk"]
    bank = meta["bank"]
    S0s, S1s, OFF = meta["S0s"], meta["S1s"], meta["OFF"]
    TOT, SMAX = meta["TOT"], meta["SMAX"]
    NT = math.ceil(vpc / 128)

    # per-layer static dims
    H_ = {0: 4, 1: 4, 2: 1}
    FH_ = {0: 256, 1: 256, 2: 64}
    MCOL = {0: 256, 1: 260, 2: 65}  # scatter rhs width
    HG = {0: 64, 1: 65, 2: 65}  # per-head stride in ps_sc
    ALS = {0: 256, 1: 260, 2: 65}  # a_src column offset in edge rows
    ROWL = {0: 260, 1: ROW[1], 2: ROW[2]}  # edge-row stride

    nc = bacc.Bacc(trn_type="TRN2", num_devices=ncores,
                   num_swdge_queues=N_SWDGE_Q,
                   dynamic_dma_scratch_size=DMA_SCRATCH)
    rg = [list(range(ncores))]

    # ---------------- I/O ----------------
    xT_in = nc.dram_tensor("xT", [128, vpc], bf16, kind="ExternalInput")
    eidx_in = nc.dram_tensor("eidx", [128, TOT * 8], i16, kind="ExternalInput")
    sa_in = nc.dram_tensor("sa", [128, TOT * 128], bf16, kind="ExternalInput")
    saT_in = nc.dram_tensor("saT", [128, TOT * 128], bf16, kind="ExternalInput")
    xe_in = nc.dram_tensor("xe", [128, TOT * 128], bf16, kind="ExternalInput")
    out3 = nc.dram_tensor("out3", [vpc, OUT_DIM], f32, kind="ExternalOutput")

    W1aug_t = nc.inline_tensor(consts["W1aug"], "W1aug")
    W1Ad_t = nc.inline_tensor(consts["W1Ad"], "W1Ad")
    W2aug_t = nc.inline_tensor(consts["W2aug"], "W2aug")
    W3aug_t = nc.inline_tensor(consts["W3aug"], "W3aug")
    b1_t = nc.inline_tensor(consts["b1"], "b1c")
    b2_t = nc.inline_tensor(consts["b2"], "b2c")
    b3_t = nc.inline_tensor(consts["b3"], "b3c")
    ident_t = nc.inline_tensor(consts["ident"], "identc")

    # internal DRAM
    tabs_in = {li: nc.dram_tensor(f"tab{li}_in", [vpc, ROW[li]], bf16)
               for li in (1, 2)}
    tabs = {li: nc.dram_tensor(f"tab{li}", [n, ROW[li]], bf16,
                               addr_space="Shared") for li in (1, 2)}
    aldb = {li: nc.dram_tensor(f"aldb{li}", [vpc, 4], bf16) for li in (0, 1, 2)}
    x1f = nc.dram_tensor("x1f", [vpc, 256], f32)
    xT2 = nc.dram_tensor("xT2", [256, vpc], bf16)
    xT3 = nc.dram_tensor("xT3", [256, vpc], bf16)
    xT_next = {0: xT2, 1: xT3}
    lhsT_srcs = {1: xT2, 2: xT3}

    AP = bass.AP

    def rd(ap, offset_elems, dims):
        return AP(ap.tensor, ap.offset + offset_elems,
                  [list(ap.ap[0])] + [list(d) for d in dims])

    with tile.TileContext(nc) as tc:
        with (
            tc.tile_pool(name="const", bufs=1) as cpool,
            tc.tile_pool(name="p1", bufs=3) as p1,
            tc.tile_pool(name="g", bufs=4) as gp,
            tc.tile_pool(name="e", bufs=3) as ep,
            tc.tile_pool(name="small", bufs=6) as sp,
            tc.tile_pool(name="psA", bufs=2, space="PSUM") as ppA,
            tc.tile_pool(name="psB", bufs=2, space="PSUM") as ppB,
            tc.tile_pool(name="psC", bufs=2, space="PSUM") as ppC,
            tc.tile_pool(name="psumT", bufs=2, space="PSUM") as ppT,
        ):
            ident_sb = cpool.tile([128, 128], bf16, tag="ident")
            nc.sync.dma_start(ident_sb[:], ident_t[:])
            bias_sb = []
            for li, bt in enumerate([b1_t, b2_t, b3_t]):
                b_sb = cpool.tile([128, bt.shape[1]], f32, tag=f"bias{li}")
                nc.sync.dma_start(b_sb[:], bt[:])
                bias_sb.append(b_sb)
            W1aug_sb = cpool.tile([128, 260], bf16, tag="w1aug")
            nc.sync.dma_start(W1aug_sb[:], W1aug_t[:])
            W1Ad_sb = cpool.tile([128, 4], bf16, tag="w1ad")
            nc.sync.dma_start(W1Ad_sb[:], W1Ad_t[:])
            W2aug_sb = []
            for k in range(2):
                w_sb = cpool.tile([128, 268], bf16, tag=f"w2aug{k}")
                nc.sync.dma_start(w_sb[:], W2aug_t[k * 128 : (k + 1) * 128, :])
                W2aug_sb.append(w_sb)
            W3aug_sb = []
            for k in range(2):
                w_sb = cpool.tile([128, 68], bf16, tag=f"w3aug{k}")
                nc.sync.dma_start(w_sb[:], W3aug_t[k * 128 : (k + 1) * 128, :])
                W3aug_sb.append(w_sb)

            # ---------------- phase 0: aldb[0] = x @ (W1 A_d) -------------
            def p0_tile(t):
                nt = min(128, vpc - t * 128)
                lw = p1.tile([128, 128], bf16, tag="lw")
                nc.sync.dma_start(lw[:, 0:nt],
                                  xT_in[:, t * 128 : t * 128 + nt])
                ps0 = ppB.tile([128, 268], f32, tag="psA")
                nc.tensor.matmul(ps0[0:nt, 0:4], lhsT=lw[:, 0:nt],
                                 rhs=W1Ad_sb[:], start=True, stop=True)
                ad_t = p1.tile([128, 4], bf16, tag="ad_t")
                nc.scalar.activation(ad_t[0:nt, :], ps0[0:nt, 0:4], Act.Copy)
                nc.sync.dma_start(aldb[0][t * 128 : t * 128 + nt, :],
                                  ad_t[0:nt, :])

            # ---------------- phase 1 (layers 2,3): node tables -----------
            def p1_tile(li, t):
                # li in (1, 2): h_aug table for layer li from xT2/xT3
                nt = min(128, vpc - t * 128)
                W_sb = W2aug_sb if li == 1 else W3aug_sb
                aug = 268 if li == 1 else 68
                used = 264 if li == 1 else 66
                row = ROW[li]
                ps1 = ppB.tile([128, 268], f32, tag="psA")
                for k in range(2):
                    lw = p1.tile([128, 128], bf16, tag="lw")
                    nc.sync.dma_start(
                        lw[:, 0:nt],
                        lhsT_srcs[li][k * 128 : (k + 1) * 128,
                                      t * 128 : t * 128 + nt])
                    nc.tensor.matmul(ps1[0:nt, 0:aug], lhsT=lw[:, 0:nt],
                                     rhs=W_sb[k][:], start=(k == 0),
                                     stop=(k == 1))
                hb = p1.tile([128, row], bf16, tag=f"hb{li}")
                nc.vector.tensor_copy(hb[0:nt, 0:used], ps1[0:nt, 0:used])
                if li == 1:
                    # per-head ones columns at 64,129,194,259
                    nc.vector.memset(rd(hb[0:nt, :], 64, [[65, 4]]), 1.0)
                    nc.vector.memset(hb[0:nt, 264:row], 0.0)
                    ad_t = p1.tile([128, 4], bf16, tag="ad_t")
                    nc.scalar.activation(ad_t[0:nt, :], ps1[0:nt, 264:268],
                                         Act.Copy)
                else:
                    nc.vector.memset(hb[0:nt, 64:65], 1.0)
                    nc.vector.memset(hb[0:nt, 66:row], 0.0)
                    ad_t = p1.tile([128, 4], bf16, tag="ad_t")
                    nc.vector.memset(ad_t[0:nt, :], 0.0)
                    nc.scalar.activation(ad_t[0:nt, 0:1], ps1[0:nt, 66:67],
                                         Act.Copy)
                nc.sync.dma_start(tabs_in[li][t * 128 : t * 128 + nt, :],
                                  hb[0:nt, :])
                nc.sync.dma_start(aldb[li][t * 128 : t * 128 + nt, :],
                                  ad_t[0:nt, :])

            def ag_chunk(li, ci):
                r0 = ci * AG_CHUNK
                k0 = ci * ncores * AG_CHUNK
                nc.gpsimd.collective_compute(
                    "AllGather",
                    Alu.bypass,
                    replica_groups=rg,
                    ins=[tabs_in[li][r0 : r0 + AG_CHUNK, :].opt()],
                    outs=[tabs[li][k0 : k0 + ncores * AG_CHUNK, :].opt()],
                )

            # ---------------- phase 2: edge blocks ------------------------
            def p2_block(li, b):
                H = H_[li]
                FH = FH_[li]
                mcol = MCOL[li]
                hg = HG[li]
                als_off = ALS[li]
                rowl = ROWL[li]
                S0, S1 = S0s[b], S1s[b]
                S = S0 + S1
                co = OFF[b]

                sa_sb = ep.tile([128, SMAX * 128], bf16, tag="sa")
                nc.sync.dma_start(sa_sb[:, 0 : S * 128],
                                  sa_in[:, co * 128 : (co + S) * 128])
                saT_sb = ep.tile([128, SMAX * 128], bf16, tag="saT")
                nc.sync.dma_start(saT_sb[:, 0 : S * 128],
                                  saT_in[:, co * 128 : (co + S) * 128])
                aldb_sb = sp.tile([128, 4], bf16, tag="aldb")
                nc.vector.memset(aldb_sb[:], 0.0)
                nc.sync.dma_start(aldb_sb[0:blk, :],
                                  aldb[li][b * blk : (b + 1) * blk, :])

                if li == 0:
                    xe_sb = ep.tile([128, SMAX * 128], bf16, tag="xe")
                    nc.sync.dma_start(xe_sb[:, 0 : S * 128],
                                      xe_in[:, co * 128 : (co + S) * 128])
                    he = ep.tile([128, SMAX, 260], bf16, tag="he")
                    for j in range(S):
                        ph = ppB.tile([128, 268], f32, tag="psA")
                        nc.tensor.matmul(
                            ph[:, 0:260],
                            lhsT=xe_sb[:, j * 128 : (j + 1) * 128],
                            rhs=W1aug_sb[:], start=True, stop=True)
                        nc.scalar.activation(he[:, j, :], ph[:, 0:260],
                                             Act.Copy)
                    src_t = he
                else:
                    eix = sp.tile([128, SMAX * 8], i16, tag="eidx")
                    nc.sync.dma_start(eix[:, 0 : S * 8],
                                      eidx_in[:, co * 8 : (co + S) * 8])
                    g1 = gp.tile([128, SMAX, ROW[li]], bf16, tag="g1")

                    def gath(slot0, nslots, tab_ap):
                        total = nslots * 128
                        for c0 in range(0, total, GATHER_CHUNK):
                            cn = min(GATHER_CHUNK, total - c0)
                            s0 = slot0 + c0 // 128
                            i0 = slot0 * 8 + c0 // 16
                            nc.gpsimd.dma_gather(
                                g1[:, s0 : s0 + cn // 128, :],
                                tab_ap,
                                eix[:, i0 : i0 + cn // 16],
                                cn, cn, ROW[li],
                                queue_num=b % N_SWDGE_Q)

                    gath(0, S0, tabs[li][0:bank, :])
                    gath(S0, S1, tabs[li][bank:n, :])
                    src_t = g1

                # ---- a_dst expansion: dst-local -> per-edge via saT
                alp = ppC.tile([128, SMAX * 4], f32, tag="alp")
                for j in range(S):
                    nc.tensor.matmul(
                        alp[:, j * H : (j + 1) * H],
                        lhsT=saT_sb[:, j * 128 : (j + 1) * 128],
                        rhs=aldb_sb[:, 0:H], start=True, stop=True)
                alf = sp.tile([128, SMAX * 4], f32, tag="alf")
                nc.scalar.activation(alf[:, 0 : S * H], alp[:, 0 : S * H],
                                     Act.Copy)

                # ---- logits -> exp(leaky) = max(exp(0.2 s), exp(s))
                t0 = sp.tile([128, SMAX * 4], f32, tag="t0")
                nc.vector.tensor_tensor(
                    out=t0[:, 0 : S * H],
                    in0=rd(src_t[:], als_off, [[rowl, S], [1, H]]),
                    in1=alf[:, 0 : S * H], op=Alu.add)
                e1 = sp.tile([128, SMAX * 4], f32, tag="e1")
                nc.scalar.activation(e1[:, 0 : S * H], t0[:, 0 : S * H],
                                     Act.Exp, scale=NEG_SLOPE)
                e2 = sp.tile([128, SMAX * 4], f32, tag="e2")
                nc.scalar.activation(e2[:, 0 : S * H], t0[:, 0 : S * H],
                                     Act.Exp)
                exb = sp.tile([128, SMAX * 4], bf16, tag="exb")
                nc.vector.tensor_tensor(
                    out=exb[:, 0 : S * H], in0=e1[:, 0 : S * H],
                    in1=e2[:, 0 : S * H], op=Alu.max)

                # ---- m = h_src * ex (per-head broadcast over C)
                m = ep.tile([128, SMAX, MCOL[li]], bf16, tag="m")
                if li == 0:
                    nc.vector.tensor_tensor(
                        out=rd(m[:], 0, [[256, S], [64, 4], [1, 64]]),
                        in0=rd(src_t[:], 0, [[260, S], [64, 4], [1, 64]]),
                        in1=rd(exb[:], 0, [[4, S], [1, 4], [0, 64]]),
                        op=Alu.mult)
                elif li == 1:
                    nc.vector.tensor_tensor(
                        out=rd(m[:], 0, [[260, S], [65, 4], [1, 65]]),
                        in0=rd(src_t[:], 0, [[384, S], [65, 4], [1, 65]]),
                        in1=rd(exb[:], 0, [[4, S], [1, 4], [0, 65]]),
                        op=Alu.mult)
                else:
                    nc.vector.tensor_tensor(
                        out=rd(m[:], 0, [[65, S], [1, 65]]),
                        in0=rd(src_t[:], 0, [[128, S], [1, 65]]),
                        in1=rd(exb[:], 0, [[1, S], [0, 65]]),
                        op=Alu.mult)

                # ---- scatter-add one-hot matmuls
                ps = ppA.tile([128, 260], f32, tag="ps_sc")
                for j in range(S):
                    nc.tensor.matmul(
                        ps[0:blk, 0 : MCOL[li]],
                        lhsT=sa_sb[:, j * 128 : j * 128 + blk],
                        rhs=m[:, j, :], start=(j == 0), stop=(j == S - 1))
                if li == 0:
                    for j in range(S):
                        nc.tensor.matmul(
                            ps[0:blk, 256:260],
                            lhsT=sa_sb[:, j * 128 : j * 128 + blk],
                            rhs=exb[:, j * 4 : (j + 1) * 4],
                            start=(j == 0), stop=(j == S - 1))

                # ---- epilogue
                rec = sp.tile([128, 4], f32, tag="rec")
                if li == 0:
                    nc.vector.reciprocal(rec[0:blk, :], ps[0:blk, 256:260])
                elif li == 1:
                    nc.vector.reciprocal(rec[0:blk, 0:4],
                                         rd(ps[0:blk, :], 64, [[65, 4]]))
                else:
                    nc.vector.reciprocal(rec[0:blk, 0:1], ps[0:blk, 64:65])
                o = sp.tile([128, 256], f32, tag="o")
                for h in range(H):
                    nc.scalar.activation(
                        o[0:blk, h * 64 : (h + 1) * 64],
                        ps[0:blk, h * hg : h * hg + 64],
                        Act.Copy, scale=rec[0:blk, h : h + 1])
                nc.vector.tensor_tensor(
                    out=o[0:blk, 0:FH], in0=o[0:blk, 0:FH],
                    in1=bias_sb[li][0:blk, 0:FH], op=Alu.add)
                if li < 2:
                    nc.vector.tensor_scalar_max(o[0:blk, 0:FH],
                                                o[0:blk, 0:FH], 0.0)
                if li == 1:
                    xr = sp.tile([128, 256], f32, tag="xr")
                    nc.sync.dma_start(xr[0:blk, :],
                                      x1f[b * blk : (b + 1) * blk, :])
                    nc.vector.tensor_tensor(out=o[0:blk, 0:FH],
                                            in0=o[0:blk, 0:FH],
                                            in1=xr[0:blk, :], op=Alu.add)
                if li == 2:
                    nc.sync.dma_start(out3[b * blk : (b + 1) * blk, :],
                                      o[0:blk, 0:64])
                    return
                if li == 0:
                    nc.sync.dma_start(x1f[b * blk : (b + 1) * blk, :],
                                      o[0:blk, 0:FH])
                ob = sp.tile([128, 256], bf16, tag="ob")
                nc.scalar.activation(ob[0:blk, :], o[0:blk, 0:256], Act.Copy)
                for c2 in range(2):
                    pt = ppT.tile([128, 128], bf16, tag="pt")
                    nc.tensor.transpose(
                        pt[:, 0:blk], ob[0:blk, c2 * 128 : (c2 + 1) * 128],
                        ident_sb[0:blk, 0:blk])
                    st = sp.tile([128, 128], bf16, tag="st")
                    nc.vector.tensor_copy(st[:, 0:blk], pt[:, 0:blk])
                    nc.sync.dma_start(
                        xT_next[li][c2 * 128 : (c2 + 1) * 128,
                                    b * blk : (b + 1) * blk],
                        st[:, 0:blk])

            # ------------- interleaved emission schedule -----------------
            NCHUNK = vpc // AG_CHUNK

            def tiles_ready_after_block(b):
                out = []
                for t in range(NT):
                    nt = min(128, vpc - t * 128)
                    breq = min(nblk - 1, (t * 128 + nt - 1) // blk)
                    if breq == b:
                        out.append(t)
                return out

            def ags_ready_after_tile(t):
                out = []
                for ci in range(NCHUNK):
                    treq = min(NT - 1, (ci * AG_CHUNK + AG_CHUNK - 1) // 128)
                    if treq == t:
                        out.append(ci)
                return out

            for t in range(NT):
                p0_tile(t)
            for li in range(3):
                for b in range(nblk):
                    p2_block(li, b)
                    if li < 2:
                        for t in tiles_ready_after_block(b):
                            p1_tile(li + 1, t)
                            for ci in ags_ready_after_tile(t):
                                ag_chunk(li + 1, ci)
    return nc


# ---------------------------------------------------------------- runner
def _run(per_core, consts, meta, sim=False, trace=False):
    from concourse.bass_utils import run_bass_kernel_spmd

    nc = build_program(meta, consts)
    nc.finalize()
    core_ids = list(range(meta["ncores"]))
    in_maps = [dict(pc) for pc in per_core]
    if sim:
        from concourse.bass_interp import MultiCoreSim

        ms = MultiCoreSim(nc, meta["ncores"])
        for c in core_ids:
            for k, v in in_maps[c].items():
                ms.cores[c].tensor(k)[:] = v
        ms.simulate()
        outs = [np.array(ms.cores[c].tensor("out3")) for c in core_ids]
        return np.concatenate(outs, axis=0), None
    res = run_bass_kernel_spmd(nc, in_maps, core_ids, trace=trace)
    global LAST_EXEC_NS, LAST_RES
    LAST_RES = res
    LAST_EXEC_NS = getattr(res, "exec_time_ns", None)
    outs = [res.results[c]["out3"] for c in core_ids]
    return np.concatenate(outs, axis=0), res


LAST_EXEC_NS = None
LAST_RES = None


def kernel(**inputs):
    x = np.asarray(inputs["x"], np.float32)
    edge_index = np.asarray(inputs["edge_index"])
    cfg = _cfg_full()
    per_core, consts, meta = build_host_data(x, edge_index, inputs, cfg)
    out, _ = _run(
        per_core, consts, meta,
        sim=bool(int(os.environ.get("GAT_SIM", "0"))),
        trace=bool(int(os.environ.get("GAT_TRACE", "0"))),
    )
    return out.astype(np.float32)
